# revision 2
# baseline (speedup 1.0000x reference)
"""nn_Attention_60266981097535 kernel — v3: runs on the 8 NeuronCores.

Sharding (per spec hint): 8 shards = (batch b in 0..3) x (query-row
half in 0..1), data-parallel via jax.pmap over the axon-tunneled trn2
NeuronCores.  Each shard computes the full per-batch q/k/R (needed
globally) and its 512-row half of the attention + output.

Algebraic optimization: diag(R) @ attn @ diag(R) row/col scaling
commutes with leaky_relu (R = sigmoid(..) > 0, leaky_relu positively
homogeneous), so R is folded into q/k before the (T,T) score matmul —
removes two O(H*T*T) elementwise passes.

Falls back to multithreaded CPU JAX if no accelerator is usable.
"""

import numpy as np

B, T, DIM, H = 4, 1024, 256, 8
D = DIM // H
HALF = T // 2
N_SHARDS = 8


def _shard_fn(shard_idx, x, adj_u8, Wq_g, Wk_g, Wv_g, Wq, Wk, Wv, Wkf,
              Wkf2, sparse_D, randomatrix):
    """x: (T, DIM) f32, adj_u8: (H, T, T) uint8 — one batch.
    Returns output rows [s0:s0+HALF], s0 = (shard_idx % 2) * HALF."""
    import jax
    import jax.numpy as jnp

    scale = DIM ** (-0.5)
    adj_f = adj_u8.astype(jnp.float32)                      # h t t

    xh = x.reshape(T, H, D).transpose(1, 0, 2)              # h t d
    xq = jnp.einsum('htd,de->hte', xh, Wq_g)
    xk = jnp.einsum('htd,de->hte', xh, Wk_g)
    q_g = jax.nn.relu(jnp.einsum('hst,htd->hsd', adj_f, xq))
    k_g = jax.nn.relu(jnp.einsum('hst,htd->hsd', adj_f, xk))

    q = q_g.transpose(1, 0, 2).reshape(T, DIM) @ Wq         # t dim
    k = k_g.transpose(1, 0, 2).reshape(T, DIM) @ Wk

    R0 = jax.nn.gelu(jnp.concatenate([q, k], axis=-1) @ Wkf,
                     approximate=False)                      # t h
    R = jnp.einsum('th,tk->hk', R0, R0)                      # h h
    R = jax.nn.sigmoid((R @ Wkf2) / sparse_D)                # h t

    qh = q.reshape(T, H, D).transpose(1, 0, 2) * R[..., None]  # h t d
    kh = k.reshape(T, H, D).transpose(1, 0, 2) * R[..., None]

    s0 = (shard_idx % 2) * HALF
    qh_half = jax.lax.dynamic_slice_in_dim(qh, s0, HALF, axis=1)

    attn = jax.nn.leaky_relu(
        jnp.einsum('hld,htd->hlt', qh_half, kh) * scale)     # h half t
    attn = jnp.einsum('lh,hst->lst', randomatrix, attn)

    adj_half = jax.lax.dynamic_slice_in_dim(adj_u8, s0, HALF, axis=1)
    attn = jnp.where(adj_half > 0, attn, jnp.asarray(-1e12, jnp.float32))
    attn = jax.nn.softmax(attn, axis=-1)                     # h half t

    xv = jnp.einsum('htd,de->hte', xh, Wv_g)
    v = jax.nn.relu(jnp.einsum('hst,htd->hsd', attn, xv))    # h half d
    out = jax.nn.gelu(v.transpose(1, 0, 2).reshape(HALF, DIM) @ Wv,
                      approximate=False)
    return out


def _run_pmap(devs, x, adj, weights):
    import jax
    import jax.numpy as jnp

    n = N_SHARDS
    adj_u8 = np.asarray(adj, np.uint8)                       # 0/1 exact
    xs = np.stack([np.asarray(x, np.float32)[i // 2] for i in range(n)])
    adjs = np.stack([adj_u8[i // 2] for i in range(n)])
    idxs = np.arange(n, dtype=np.int32)

    f = jax.pmap(
        _shard_fn,
        in_axes=(0, 0, 0) + (None,) * 10,
        devices=devs[:n],
    )
    out_halves = np.asarray(f(idxs, xs, adjs, *weights))
    out = np.empty((B, T, DIM), np.float32)
    for i in range(n):
        b, hh = i // 2, i % 2
        out[b, hh * HALF:(hh + 1) * HALF] = out_halves[i]
    return out


def _run_cpu(x, adj, weights):
    import jax
    import jax.numpy as jnp

    def full(x, adj_u8, *w):
        outs = []
        for bb in range(B):
            o0 = _shard_fn(0, x[bb], adj_u8[bb], *w)
            o1 = _shard_fn(1, x[bb], adj_u8[bb], *w)
            outs.append(jnp.concatenate([o0, o1], axis=0))
        return jnp.stack(outs)

    cpu = jax.devices('cpu')[0]
    f = jax.jit(full, backend='cpu')
    with jax.default_device(cpu):
        return np.asarray(f(np.asarray(x, np.float32),
                            np.asarray(adj, np.uint8), *weights))


def kernel(x, adj, Wq_g, Wk_g, Wv_g, Wq, Wk, Wv, Wkf, Wkf2, sparse_D,
           randomatrix, label):
    import jax

    try:
        jax.config.update("jax_compilation_cache_dir",
                          "/tmp/jax_kcache_60266981097535")
        jax.config.update("jax_persistent_cache_min_entry_size_bytes", -1)
        jax.config.update("jax_persistent_cache_min_compile_time_secs", 0)
    except Exception:
        pass

    weights = tuple(
        np.asarray(w, np.float32)
        for w in (Wq_g, Wk_g, Wv_g, Wq, Wk, Wv, Wkf, Wkf2, sparse_D,
                  randomatrix))

    try:
        devs = [d for d in jax.devices() if d.platform != 'cpu']
        if len(devs) >= N_SHARDS:
            return _run_pmap(devs, x, adj, weights)
    except Exception:
        pass
    return _run_cpu(x, adj, weights)


# revision 3
# speedup vs baseline: 58.3029x; 58.3029x over previous
"""nn_Attention_60266981097535 kernel — 8 NeuronCores via jax.pmap.

Sharding (per spec hint): 8 shards = (batch b in 0..3) x (query-row
half in 0..1), data-parallel across the 8 axon-tunneled trn2
NeuronCores.  Each shard computes the full per-batch q/k/R (needed
globally: k and the R Gram span all tokens) and its 512-row half of
the attention + output; no collectives needed.

Optimizations:
- adj is transferred as packed bits (np.packbits, 8 entries/byte):
  256 MiB of per-shard uint8 -> 32 MiB over the device link, unpacked
  on-device with shift/mask.
- diag(R) @ attn @ diag(R) commutes with leaky_relu (R = sigmoid > 0,
  leaky_relu positively homogeneous), so R is folded into q/k before
  the (T,T) score matmul — removes two O(H*T*T) elementwise passes.

Falls back to multithreaded CPU JAX if no accelerator is usable.
"""

import numpy as np

B, T, DIM, H = 4, 1024, 256, 8
D = DIM // H
HALF = T // 2
N_SHARDS = 8


def _shard_fn(shard_idx, x, adj_bits, Wq_g, Wk_g, Wv_g, Wq, Wk, Wv, Wkf,
              Wkf2, sparse_D, randomatrix):
    """x: (T, DIM) f32, adj_bits: (H, T, T//8) u8 (big-endian bit order).
    Returns output rows [s0:s0+HALF], s0 = (shard_idx % 2) * HALF."""
    import jax
    import jax.numpy as jnp

    scale = DIM ** (-0.5)
    shifts = jnp.arange(7, -1, -1, dtype=jnp.uint8)          # packbits 'big'
    adj_u8 = ((adj_bits[..., None] >> shifts) & jnp.uint8(1)).reshape(
        H, T, T)                                              # h t t 0/1
    adj_f = adj_u8.astype(jnp.float32)

    xh = x.reshape(T, H, D).transpose(1, 0, 2)                # h t d
    xq = jnp.einsum('htd,de->hte', xh, Wq_g)
    xk = jnp.einsum('htd,de->hte', xh, Wk_g)
    q_g = jax.nn.relu(jnp.einsum('hst,htd->hsd', adj_f, xq))
    k_g = jax.nn.relu(jnp.einsum('hst,htd->hsd', adj_f, xk))

    q = q_g.transpose(1, 0, 2).reshape(T, DIM) @ Wq           # t dim
    k = k_g.transpose(1, 0, 2).reshape(T, DIM) @ Wk

    R0 = jax.nn.gelu(jnp.concatenate([q, k], axis=-1) @ Wkf,
                     approximate=False)                        # t h
    R = jnp.einsum('th,tk->hk', R0, R0)                        # h h
    R = jnp.asarray(jax.nn.sigmoid((R @ Wkf2) / sparse_D))     # h t

    qh = q.reshape(T, H, D).transpose(1, 0, 2) * R[..., None]  # h t d
    kh = k.reshape(T, H, D).transpose(1, 0, 2) * R[..., None]

    s0 = (shard_idx % 2) * HALF
    qh_half = jax.lax.dynamic_slice_in_dim(qh, s0, HALF, axis=1)

    attn = jax.nn.leaky_relu(
        jnp.einsum('hld,htd->hlt', qh_half, kh) * scale)       # h half t
    attn = jnp.einsum('lh,hst->lst', randomatrix, attn)

    adj_half = jax.lax.dynamic_slice_in_dim(adj_u8, s0, HALF, axis=1)
    attn = jnp.where(adj_half > 0, attn, jnp.asarray(-1e12, jnp.float32))
    attn = jax.nn.softmax(attn, axis=-1)                       # h half t

    xv = jnp.einsum('htd,de->hte', xh, Wv_g)
    v = jax.nn.relu(jnp.einsum('hst,htd->hsd', attn, xv))      # h half d
    out = jax.nn.gelu(v.transpose(1, 0, 2).reshape(HALF, DIM) @ Wv,
                      approximate=False)
    return out


def _run_pmap(devs, x32, adj_bits, weights):
    import jax

    n = N_SHARDS
    devs = devs[:n]
    f = jax.pmap(
        _shard_fn,
        in_axes=(0, 0, 0) + (None,) * 10,
        devices=devs,
    )
    idxs = jax.device_put_sharded([np.int32(i) for i in range(n)], devs)
    xs = jax.device_put_sharded([x32[i // 2] for i in range(n)], devs)
    adjs = jax.device_put_sharded(
        [adj_bits[i // 2] for i in range(n)], devs)
    out_halves = np.asarray(f(idxs, xs, adjs, *weights))
    out = np.empty((B, T, DIM), np.float32)
    for i in range(n):
        b, hh = i // 2, i % 2
        out[b, hh * HALF:(hh + 1) * HALF] = out_halves[i]
    return out


def _run_cpu(x32, adj_bits, weights):
    import jax

    cpu = jax.devices('cpu')[0]
    fj = jax.jit(_shard_fn, static_argnums=(0,), backend='cpu')
    with jax.default_device(cpu):
        outs = []
        for bb in range(B):
            o0 = fj(0, x32[bb], adj_bits[bb], *weights)
            o1 = fj(1, x32[bb], adj_bits[bb], *weights)
            outs.append(np.concatenate([np.asarray(o0), np.asarray(o1)], 0))
    return np.stack(outs).astype(np.float32)


def kernel(x, adj, Wq_g, Wk_g, Wv_g, Wq, Wk, Wv, Wkf, Wkf2, sparse_D,
           randomatrix, label):
    import jax

    try:
        jax.config.update("jax_compilation_cache_dir",
                          "/tmp/jax_kcache_60266981097535")
        jax.config.update("jax_persistent_cache_min_entry_size_bytes", -1)
        jax.config.update("jax_persistent_cache_min_compile_time_secs", 0)
    except Exception:
        pass

    weights = tuple(
        np.asarray(w, np.float32)
        for w in (Wq_g, Wk_g, Wv_g, Wq, Wk, Wv, Wkf, Wkf2, sparse_D,
                  randomatrix))
    x32 = np.asarray(x, np.float32)
    adj_bits = np.packbits(np.asarray(adj, np.uint8), axis=-1)  # B H T T/8

    try:
        devs = [d for d in jax.devices() if d.platform != 'cpu']
        if len(devs) >= N_SHARDS:
            return _run_pmap(devs, x32, adj_bits, weights)
    except Exception:
        pass
    return _run_cpu(x32, adj_bits, weights)


# revision 4
# speedup vs baseline: 63.2373x; 1.0846x over previous
"""nn_Attention_60266981097535 kernel — 8 NeuronCores via jax.pmap.

Sharding (per spec hint): 8 shards = (batch b in 0..3) x (query-row
half in 0..1), data-parallel across the 8 axon-tunneled trn2
NeuronCores.  Each shard computes the full per-batch q/k/R (needed
globally: k and the R Gram span all tokens) and its 512-row half of
the attention + output; no collectives needed.

Optimizations:
- adj is transferred as packed bits (np.packbits, 8 entries/byte):
  256 MiB of per-shard uint8 -> 32 MiB over the device link, unpacked
  on-device with shift/mask.
- diag(R) @ attn @ diag(R) commutes with leaky_relu (R = sigmoid > 0,
  leaky_relu positively homogeneous), so R is folded into q/k before
  the (T,T) score matmul — removes two O(H*T*T) elementwise passes.

Falls back to multithreaded CPU JAX if no accelerator is usable.
"""

import numpy as np

B, T, DIM, H = 4, 1024, 256, 8
D = DIM // H
HALF = T // 2
N_SHARDS = 8


def _shard_fn(shard_idx, x, adj_bits, Wq_g, Wk_g, Wv_g, Wq, Wk, Wv, Wkf,
              Wkf2, sparse_D, randomatrix):
    """x: (T, DIM) f32, adj_bits: (H, T, T//8) u8 (big-endian bit order).
    Returns output rows [s0:s0+HALF], s0 = (shard_idx % 2) * HALF."""
    import jax
    import jax.numpy as jnp

    scale = DIM ** (-0.5)
    shifts = jnp.arange(7, -1, -1, dtype=jnp.uint8)          # packbits 'big'
    adj_u8 = ((adj_bits[..., None] >> shifts) & jnp.uint8(1)).reshape(
        H, T, T)                                              # h t t 0/1
    adj_f = adj_u8.astype(jnp.float32)

    xh = x.reshape(T, H, D).transpose(1, 0, 2)                # h t d
    xq = jnp.einsum('htd,de->hte', xh, Wq_g)
    xk = jnp.einsum('htd,de->hte', xh, Wk_g)
    q_g = jax.nn.relu(jnp.einsum('hst,htd->hsd', adj_f, xq))
    k_g = jax.nn.relu(jnp.einsum('hst,htd->hsd', adj_f, xk))

    q = q_g.transpose(1, 0, 2).reshape(T, DIM) @ Wq           # t dim
    k = k_g.transpose(1, 0, 2).reshape(T, DIM) @ Wk

    R0 = jax.nn.gelu(jnp.concatenate([q, k], axis=-1) @ Wkf,
                     approximate=False)                        # t h
    R = jnp.einsum('th,tk->hk', R0, R0)                        # h h
    R = jnp.asarray(jax.nn.sigmoid((R @ Wkf2) / sparse_D))     # h t

    qh = q.reshape(T, H, D).transpose(1, 0, 2) * R[..., None]  # h t d
    kh = k.reshape(T, H, D).transpose(1, 0, 2) * R[..., None]

    s0 = (shard_idx % 2) * HALF
    qh_half = jax.lax.dynamic_slice_in_dim(qh, s0, HALF, axis=1)

    attn = jax.nn.leaky_relu(
        jnp.einsum('hld,htd->hlt', qh_half, kh) * scale)       # h half t
    attn = jnp.einsum('lh,hst->lst', randomatrix, attn)

    adj_half = jax.lax.dynamic_slice_in_dim(adj_u8, s0, HALF, axis=1)
    attn = jnp.where(adj_half > 0, attn, jnp.asarray(-1e12, jnp.float32))
    attn = jax.nn.softmax(attn, axis=-1)                       # h half t

    xv = jnp.einsum('htd,de->hte', xh, Wv_g)
    v = jax.nn.relu(jnp.einsum('hst,htd->hsd', attn, xv))      # h half d
    out = jax.nn.gelu(v.transpose(1, 0, 2).reshape(HALF, DIM) @ Wv,
                      approximate=False)
    return out


def _run_pmap(devs, x32, adj_bits, weights):
    import jax
    from concurrent.futures import ThreadPoolExecutor

    n = N_SHARDS
    devs = devs[:n]
    f = jax.pmap(
        _shard_fn,
        in_axes=(0, 0, 0) + (None,) * 10,
        devices=devs,
    )
    idxs = jax.device_put_sharded([np.int32(i) for i in range(n)], devs)
    # Per-device puts from threads overlap the tunnel's transfer latency
    # (~1.4x faster than a serial device_put_sharded of host arrays).
    with ThreadPoolExecutor(n) as ex:
        futs = [ex.submit(jax.device_put,
                          (x32[i // 2], adj_bits[i // 2]), devs[i])
                for i in range(n)]
        pairs = [fu.result() for fu in futs]
    xs = jax.device_put_sharded([p[0] for p in pairs], devs)
    adjs = jax.device_put_sharded([p[1] for p in pairs], devs)
    out_halves = np.asarray(f(idxs, xs, adjs, *weights))
    out = np.empty((B, T, DIM), np.float32)
    for i in range(n):
        b, hh = i // 2, i % 2
        out[b, hh * HALF:(hh + 1) * HALF] = out_halves[i]
    return out


def _run_cpu(x32, adj_bits, weights):
    import jax

    cpu = jax.devices('cpu')[0]
    fj = jax.jit(_shard_fn, static_argnums=(0,), backend='cpu')
    with jax.default_device(cpu):
        outs = []
        for bb in range(B):
            o0 = fj(0, x32[bb], adj_bits[bb], *weights)
            o1 = fj(1, x32[bb], adj_bits[bb], *weights)
            outs.append(np.concatenate([np.asarray(o0), np.asarray(o1)], 0))
    return np.stack(outs).astype(np.float32)


def kernel(x, adj, Wq_g, Wk_g, Wv_g, Wq, Wk, Wv, Wkf, Wkf2, sparse_D,
           randomatrix, label):
    import jax

    try:
        jax.config.update("jax_compilation_cache_dir",
                          "/tmp/jax_kcache_60266981097535")
        jax.config.update("jax_persistent_cache_min_entry_size_bytes", -1)
        jax.config.update("jax_persistent_cache_min_compile_time_secs", 0)
    except Exception:
        pass

    weights = tuple(
        np.asarray(w, np.float32)
        for w in (Wq_g, Wk_g, Wv_g, Wq, Wk, Wv, Wkf, Wkf2, sparse_D,
                  randomatrix))
    x32 = np.asarray(x, np.float32)
    adj_bits = np.packbits(np.asarray(adj, np.uint8), axis=-1)  # B H T T/8

    try:
        devs = [d for d in jax.devices() if d.platform != 'cpu']
        if len(devs) >= N_SHARDS:
            return _run_pmap(devs, x32, adj_bits, weights)
    except Exception:
        pass
    return _run_cpu(x32, adj_bits, weights)
